# revision 68
# baseline (speedup 1.0000x reference)
"""DualAttention2d Trainium2 kernel.

Sharding: 8 cores = 4 samples x {spatial-attention branch, channel-attention
branch}. Core c < 4 computes the spatial branch of sample c; core c >= 4
computes the channel branch of sample c-4. Host sums the two branch outputs.

Single SPMD program; branch divergence via tc.If(partition_id < 4).

Layout notes:
- Feature maps on-chip as [4 blocks][128 chan, S] with S = 64*64 = 4096.
- Conv inputs live in a zero-padded [128, 66*66] buffer (1-px halo); a 3x3
  conv is 9 shifted matmuls accumulated in PSUM over 4 channel blocks, two
  512-wide s-tiles per [128,1024] PSUM pair-tile, conv weights SBUF-resident
  for the whole conv (loaded once, split across DMA queues).
- BN is folded into conv weights/bias on the host; alpha into the v-proj.
- f32r matmuls run at full PE rate (free dim >= 512). The o = v @ att^T
  matmul runs in fp8e4 DoubleRow (256-deep k pairs, 2x PE rate): vT is
  produced fp8 directly by the conv1-fused v projection and kept SBUF-
  resident; attention probabilities (bf16, unnormalized) are transposed by
  PE in 4x[128,128] batches and evicted to the fp8 attT by DVE/Act.
  End-to-end fp8 error ~1.7e-3 (tolerance 2e-2).
- Softmax: logits pairs staged PSUM->SBUF f32 (copies split DVE/Act),
  per-pair row-max on DVE, single exp per 128-query block on Act with
  accumulated row-sum. Probabilities stay UNNORMALIZED; 1/rowsum is folded
  into the o eviction via a DRAM-roundtrip transpose+partition-broadcast
  of the 4 recip columns, applied as a [128,512] columnwise multiply, with
  the v-bias and residual fused in one scalar_tensor_tensor.
- Software pipelining: PE transposes of block b-1 are emitted between the
  logits matmuls of block b, and the o-matmuls of group g-1 are emitted
  under the early softmax blocks of group g, so PE never waits on the
  softmax chain. Startup: only conv pair 0's input rows load up front; the
  bulk of xpad queues behind the conv weight loads.
- HW quirks honored: GpSimd/Pool cannot touch PSUM (evictions on DVE/Act
  only); f32r cannot mix with bf16 matmul operands; DMA-issue SEQ slices
  span the transfer (big DMAs split across SP/Pool queues).
- TimelineSim: spatial ~0.99 ms, channel ~0.70 ms per core (baseline
  1.38/0.72); measured HW rel err 1.67e-3.
"""

import numpy as np

import concourse.bacc as bacc
import concourse.mybir as mybir
import concourse.tile as tile
from concourse.bass_utils import run_bass_kernel_spmd

B, C, H, W = 4, 512, 64, 64
S = H * W            # 4096
CI = 64              # q/k channels
P = 128
NB = C // P          # 4 channel blocks
PW = 66              # padded row width
PR = 66              # padded rows (1 zero row top/bottom)
PAD = PW * PR        # 4356
NST = S // 512       # 8 s-tiles of 512
NCH = S // P         # 32 s-chunks of 128
EPS = 1e-5

F32 = mybir.dt.float32
F32R = mybir.dt.float32r
BF16 = mybir.dt.bfloat16
F8 = mybir.dt.float8e4
DR = mybir.MatmulPerfMode.DoubleRow
AF = mybir.ActivationFunctionType
AX = mybir.AxisListType

_CACHE = {}


def _pad_view(xpad_ap, st, dy=1, dx=1):
    """View of padded buffer [128, PAD] covering s-tile `st` (8 image rows x 64
    cols) shifted by tap (dy, dx) in {0,1,2}^2. dy=dx=1 is the centered view."""
    v = xpad_ap.rearrange("p (r w) -> p r w", w=PW)
    r0 = st * 8 + dy
    return v[:, r0:r0 + 8, dx:dx + 64]


def build(branch=None, reps=1):
    """branch=None: SPMD program with If/Else on partition id.
    branch="spatial"/"channel": single-branch program (analysis/timing).
    reps>1 repeats the whole computation (for HW timing: per-rep exec =
    (wall[R] - wall[1]) / (R - 1), subtracting dispatch overhead)."""
    nc = bacc.Bacc("TRN2", target_bir_lowering=False, debug=False,
                   num_devices=8)

    # ---- I/O ----
    x_d = nc.dram_tensor("xpad", [NB, P, PAD], F32R, kind="ExternalInput")
    # conv weights pre-arranged host-side: [ob, tap, cb, ci, o]
    w1_d = nc.dram_tensor("w1", [NB, 36, P, P], F32R, kind="ExternalInput")
    b1_d = nc.dram_tensor("b1", [NB, P, 1], F32, kind="ExternalInput")
    w2_d = nc.dram_tensor("w2", [NB, 36, P, P], F32R, kind="ExternalInput")
    b2_d = nc.dram_tensor("b2", [NB, P, 1], F32, kind="ExternalInput")
    qw_d = nc.dram_tensor("qw", [NB, P, CI], F32R, kind="ExternalInput")
    kw_d = nc.dram_tensor("kw", [NB, P, CI], F32R, kind="ExternalInput")
    vw_d = nc.dram_tensor("vw", [NB, P, 512], F32R, kind="ExternalInput")
    qb_d = nc.dram_tensor("qb", [CI, 1], F32, kind="ExternalInput")
    kb_d = nc.dram_tensor("kb", [CI, 1], F32, kind="ExternalInput")
    vba_d = nc.dram_tensor("vba", [NB, P, 1], F32, kind="ExternalInput")
    beta_d = nc.dram_tensor("betat", [P, 1], F32, kind="ExternalInput")
    idr_d = nc.dram_tensor("identr", [P, P], F32R, kind="ExternalInput")
    idb_d = nc.dram_tensor("identb", [P, P], BF16, kind="ExternalInput")
    out_d = nc.dram_tensor("out", [NB, P, S], F32, kind="ExternalOutput")

    # ---- internal DRAM scratch ----
    s1_d = nc.dram_tensor("s1f", [NB, P, S], F32R, kind="Internal")
    c1t_d = nc.dram_tensor("c1t", [NCH, P, 512], F32R, kind="Internal")
    q_d = nc.dram_tensor("qs", [CI, S], F32R, kind="Internal")
    r_d = nc.dram_tensor("rcp", [2, 4, P], F32R, kind="Internal")

    with tile.TileContext(nc) as tc:
        from contextlib import ExitStack

        # ---- global pools (whole kernel) ----
        gctx = ExitStack()
        psA = gctx.enter_context(tc.tile_pool(name="psA", bufs=2,
                                              space="PSUM"))
        psL = gctx.enter_context(tc.tile_pool(name="psL", bufs=2,
                                              space="PSUM"))
        xpadp = gctx.enter_context(tc.tile_pool(name="xpadp", bufs=NB))
        consts = gctx.enter_context(tc.tile_pool(name="consts", bufs=1))
        b512 = gctx.enter_context(tc.tile_pool(name="b512", bufs=2))
        statp = gctx.enter_context(tc.tile_pool(name="statp", bufs=12))

        # ---- constants ----
        ident_r = consts.tile([P, P], F32R, name="ident_r")
        nc.sync.dma_start(ident_r[:], idr_d.ap())
        ident_b = consts.tile([P, P], BF16, name="ident_b")
        nc.sync.dma_start(ident_b[:], idb_d.ap())
        b1_t = [consts.tile([P, 1], F32, name=f"b1{i}") for i in range(NB)]
        b2_t = [consts.tile([P, 1], F32, name=f"b2{i}") for i in range(NB)]
        vba_t = [consts.tile([P, 1], F32, name=f"vba{i}") for i in range(NB)]
        qb_t = consts.tile([CI, 1], F32, name="qbt")
        kb_t = consts.tile([CI, 1], F32, name="kbt")
        beta_t = consts.tile([P, 1], F32, name="betat_sb")
        for i in range(NB):
            nc.sync.dma_start(b1_t[i][:], b1_d[i])
            nc.sync.dma_start(b2_t[i][:], b2_d[i])
            nc.sync.dma_start(vba_t[i][:], vba_d[i])
        nc.sync.dma_start(qb_t[:], qb_d.ap())
        nc.sync.dma_start(kb_t[:], kb_d.ap())
        nc.sync.dma_start(beta_t[:], beta_d.ap())

        # ---- padded input (loaded per rep) ----
        xpad = []

        def load_xpad():
            # only the rows conv pair 0 needs; bulk deferred behind the
            # conv weight loads (load_xpad_bulk) so PE starts ~10us in
            xpad.clear()
            xpad.extend(xpadp.tile([P, PAD], F32R, tag="xp", name=f"xpad{i}")
                        for i in range(NB))
            cut0 = 18 * PW
            for i in range(NB):
                (nc.sync, nc.gpsimd)[i % 2].dma_start(
                    xpad[i][:, :cut0], x_d[i, :, :cut0])

        def load_xpad_bulk():
            cut0 = 18 * PW
            cut1 = 34 * PW
            cut2 = 50 * PW
            for i in range(NB):
                nc.sync.dma_start(xpad[i][:, cut0:cut1], x_d[i, :, cut0:cut1])
                nc.gpsimd.dma_start(xpad[i][:, cut1:cut2],
                                    x_d[i, :, cut1:cut2])
                nc.sync.dma_start(xpad[i][:, cut2:], x_d[i, :, cut2:])

        def load_wres(wpool, w_dram, ob, eng=None):
            """The 36 [128,128] stationaries of one conv output block."""
            wres = wpool.tile([P, 36 * P], F32R, tag="wres", name="wres")
            (eng or nc.sync).dma_start(
                wres[:].rearrange("p (k o) -> p k o", o=P),
                w_dram[ob].rearrange("k p o -> p k o"))
            return wres

        def conv1_pair(wres, ob, st0, bounce, b1ref):
            """One conv over s-tiles (st0, st0+1) for output block ob; returns
            the evicted [128,1024] relu tile; also writes s1_d."""
            ps = psL.tile([P, 1024], F32, tag="lg", name="c1p")
            for tci in range(36):
                cb, tap = tci // 9, tci % 9
                dy, dx = tap // 3, tap % 3
                for sl in range(2):
                    nc.tensor.matmul(
                        ps[:, sl * 512:(sl + 1) * 512],
                        wres[:, tci * P:(tci + 1) * P],
                        _pad_view(xpad[cb][:], st0 + sl, dy, dx),
                        start=(tci == 0), stop=(tci == 35))
            sb = bounce.tile([P, 1024], F32R, tag="bn", name=f"sb{ob}")
            nc.scalar.activation(sb[:], ps[:], AF.Relu, bias=b1ref[ob][:])
            nc.gpsimd.dma_start(
                s1_d[ob, :, st0 * 512:(st0 + 2) * 512], sb[:])
            return sb

        def c1t_out(sb, ob, st0, tb4):
            """Transpose the pair-tile into c1t_d chunks (8 chunks)."""
            for sl in range(2):
                tb = tb4.tile([P, 512], F32R, tag="t4", name="tb")
                pt = psA.tile([P, 512], F32R, tag="mm", name="ptc")
                for j in range(4):
                    nc.tensor.transpose(
                        pt[:, j * P:(j + 1) * P],
                        sb[:, sl * 512 + j * P:sl * 512 + (j + 1) * P],
                        ident_r[:])
                nc.scalar.activation(tb[:], pt[:], AF.Identity)
                st = st0 + sl
                nc.gpsimd.dma_start(
                    c1t_d.ap()[st * 4:st * 4 + 4, :, ob * P:(ob + 1) * P]
                    .rearrange("j p c -> p j c"),
                    tb[:].rearrange("p (j c) -> p j c", c=P))

        def spatial_middle():
            # long-lived attention inputs: k global (f32r) + vT global (fp8)
            resctx = ExitStack()
            kqp = resctx.enter_context(tc.tile_pool(name="kqp", bufs=1))
            kg = kqp.tile([CI, S], F32R, tag="kg", name="kg")
            vt_sb = kqp.tile([P, NCH, 512], F8, tag="vt", name="vt_sb")

            # ---- conv1 fused with q/k/vT production, st-pair outer ----
            with ExitStack() as c1ctx:
                wp = c1ctx.enter_context(tc.tile_pool(name="wp1", bufs=4))
                bounce = c1ctx.enter_context(tc.tile_pool(name="bn1", bufs=4))
                qkvp = c1ctx.enter_context(tc.tile_pool(name="qkvp", bufs=1))
                qw_t = [qkvp.tile([P, CI], F32R, tag=f"qw{i}", name=f"qw{i}")
                        for i in range(NB)]
                kw_t = [qkvp.tile([P, CI], F32R, tag=f"kw{i}", name=f"kw{i}")
                        for i in range(NB)]
                vw_t = [qkvp.tile([P, 512], F32R, tag=f"vw{i}",
                                  name=f"vw{i}") for i in range(NB)]
                for i in range(NB):
                    nc.gpsimd.dma_start(qw_t[i][:], qw_d[i])
                    nc.gpsimd.dma_start(kw_t[i][:], kw_d[i])
                    nc.gpsimd.dma_start(vw_t[i][:], vw_d[i])
                wres4 = [load_wres(wp, w1_d.ap(), ob,
                                   (nc.sync, nc.gpsimd)[ob % 2])
                         for ob in range(NB)]
                load_xpad_bulk()
                for pair in range(NST // 2):
                    st0 = pair * 2
                    sbs = []
                    for ob in range(NB):
                        sb = conv1_pair(wres4[ob], ob, st0, bounce, b1_t)
                        sbs.append(sb)
                    # q, k, vT for the two s-tiles of this pair
                    for sl in range(2):
                        st = st0 + sl
                        ssl = slice(sl * 512, (sl + 1) * 512)
                        pq = psA.tile([CI, 512], F32, tag="mm", name="pq")
                        pk = psA.tile([CI, 512], F32, tag="mm", name="pk")
                        for cb in range(NB):
                            nc.tensor.matmul(pq[:], qw_t[cb][:],
                                             sbs[cb][:, ssl],
                                             start=(cb == 0),
                                             stop=(cb == NB - 1))
                        for cb in range(NB):
                            nc.tensor.matmul(pk[:], kw_t[cb][:],
                                             sbs[cb][:, ssl],
                                             start=(cb == 0),
                                             stop=(cb == NB - 1))
                        qsb = b512.tile([CI, 512], F32R, tag="bn",
                                        name="qsb")
                        nc.scalar.activation(qsb[:], pq[:], AF.Identity,
                                             bias=qb_t[:])
                        nc.gpsimd.dma_start(
                            q_d.ap()[:, st * 512:(st + 1) * 512], qsb[:])
                        nc.scalar.activation(kg[:, st * 512:(st + 1) * 512],
                                             pk[:], AF.Identity, bias=kb_t[:])
                        for j in range(4):
                            pv = psA.tile([P, 512], F32, tag="mm", name="pv")
                            for cb in range(NB):
                                nc.tensor.matmul(
                                    pv[:],
                                    sbs[cb][:, sl * 512 + j * P:
                                            sl * 512 + (j + 1) * P],
                                    vw_t[cb][:], start=(cb == 0),
                                    stop=(cb == NB - 1))
                            nc.scalar.activation(
                                vt_sb[:, st * 4 + j, :], pv[:], AF.Identity)

            # ---- attention, one group of 512 query positions at a time ----
            with ExitStack() as attctx:
                qgp = attctx.enter_context(tc.tile_pool(name="qgp", bufs=1))

                logp = attctx.enter_context(tc.tile_pool(name="logp", bufs=2))
                probp = attctx.enter_context(tc.tile_pool(name="probp",
                                                          bufs=2))
                attTp = attctx.enter_context(tc.tile_pool(name="attTp",
                                                          bufs=2))
                rcbp = attctx.enter_context(tc.tile_pool(name="rcbp", bufs=2))
                s1rp = attctx.enter_context(tc.tile_pool(name="s1rp", bufs=2))
                def o_phase(attT, recipb, g):
                    # o = vT^T @ attT via fp8 DoubleRow (256-deep k pairs)
                    for half in range(2):
                        cbs = (2 * half, 2 * half + 1)
                        s1r = s1rp.tile([P, 1024], F32R, tag="s1r",
                                        name="s1r")
                        nc.sync.dma_start(
                            s1r[:].rearrange("p (b n) -> p b n", n=512),
                            s1_d.ap()[2 * half:2 * half + 2, :,
                                      g * 512:(g + 1) * 512].rearrange(
                                          "b p n -> p b n"))
                        pop = psL.tile([P, 1024], F32, tag="lg", name="po")
                        po = [pop[:, :512], pop[:, 512:]]
                        for jp in range(NCH // 2):
                            for i, cb in enumerate(cbs):
                                nc.tensor.matmul(
                                    po[i],
                                    vt_sb[:, 2 * jp:2 * jp + 2,
                                          cb * P:(cb + 1) * P],
                                    attT[:, 2 * jp:2 * jp + 2, :],
                                    start=(jp == 0), stop=(jp == NCH // 2 - 1),
                                    perf_mode=DR)
                        for i, cb in enumerate(cbs):
                            ob_sb = b512.tile([P, 512], F32, tag="bn",
                                              name="obsb")
                            nc.vector.tensor_mul(ob_sb[:], po[i],
                                                 recipb[:])
                            nc.vector.scalar_tensor_tensor(
                                _pad_view(xpad[cb][:], g), ob_sb[:],
                                vba_t[cb][:], s1r[:, i * 512:(i + 1) * 512],
                                op0=mybir.AluOpType.add,
                                op1=mybir.AluOpType.add)

                prev = None
                for g in range(NST):
                    qg = qgp.tile([CI, 512], F32R, tag="qg", name="qg")
                    nc.sync.dma_start(qg[:],
                                      q_d.ap()[:, g * 512:(g + 1) * 512])
                    attT = attTp.tile([P, NCH, 512], F8, tag="attT",
                                      name=f"attT{g}")
                    recip4 = statp.tile([P, 4], F32R, tag="rc", name="recip4")
                    rowsum4 = statp.tile([P, 4], F32, tag="rs",
                                         name="rowsum4")
                    probs4 = []

                    def softmax_blk(blk):
                        """logits matmuls + softmax for one 128-query block;
                        probs left UNNORMALIZED (recip folded into o-evict)."""
                        logits = logp.tile([P, S], F32, tag="lg",
                                           name="logits")
                        for sp in range(NST // 2):
                            pl = psL.tile([P, 1024], F32, tag="lg", name="pl")
                            for sl in range(2):
                                st = sp * 2 + sl
                                nc.tensor.matmul(
                                    pl[:, sl * 512:(sl + 1) * 512],
                                    qg[:, blk * P:(blk + 1) * P],
                                    kg[:, st * 512:(st + 1) * 512],
                                    start=True, stop=True)
                            if sp == 0:
                                nc.vector.tensor_copy(
                                    logits[:, sp * 1024:(sp + 1) * 1024],
                                    pl[:])
                            else:
                                nc.scalar.copy(
                                    logits[:, sp * 1024:(sp + 1) * 1024],
                                    pl[:])
                        negmax = statp.tile([P, 1], F32, tag="st",
                                            name="negmax")
                        nc.vector.reduce_max(negmax[:], logits[:], axis=AX.X,
                                             negate=True)
                        probs = probp.tile([P, S], BF16, tag="pb",
                                           name="probs")
                        nc.scalar.activation(probs[:], logits[:], AF.Exp,
                                             bias=negmax[:],
                                             accum_out=rowsum4[:,
                                                              blk:blk + 1])
                        return probs

                    def transpose_blk(blk):
                        probs = probs4[blk]
                        for j4 in range(NCH // 4):
                            pt = psA.tile([P, 512], BF16, tag="tb",
                                          name="pt")
                            for jj in range(4):
                                j = j4 * 4 + jj
                                nc.tensor.transpose(
                                    pt[:, jj * P:(jj + 1) * P],
                                    probs[:, j * P:(j + 1) * P],
                                    ident_b[:])
                            dst = attT[:, j4 * 4:j4 * 4 + 4,
                                       blk * P:(blk + 1) * P]
                            srcv = pt[:].rearrange("p (j q) -> p j q", q=P)
                            if j4 % 2 == 0:
                                nc.vector.tensor_copy(dst, srcv)
                            else:
                                nc.scalar.copy(dst, srcv)

                    # software pipeline: transposes of blk-1 overlap blk's
                    # softmax chain on Act/DVE; the o-matmuls of g-1 fill
                    # the PE idle under the early blocks' softmax chains
                    for blk in range(4):
                        probs4.append(softmax_blk(blk))
                        if blk >= 1:
                            transpose_blk(blk - 1)
                        if blk == 1 and prev is not None:
                            o_phase(*prev)
                    transpose_blk(3)

                    # recip row via DRAM roundtrip: [128q,4] -> [1,512]
                    # transposed view -> partition-broadcast load [128,512]
                    with nc.allow_low_precision(reason="f32r==f32 bits"):
                        nc.vector.reciprocal(recip4[:], rowsum4[:])
                    nc.sync.dma_start(
                        r_d[g % 2].rearrange("b q -> q b"), recip4[:])
                    recipb = rcbp.tile([P, 512], F32R, tag="rb",
                                       name="recipb")
                    nc.sync.dma_start(
                        recipb[:],
                        r_d.ap()[g % 2:g % 2 + 1].rearrange(
                            "o b q -> o (b q)").to_broadcast((P, 512)))

                    prev = (attT, recipb, g)
                o_phase(*prev)
            resctx.close()
            conv2()

        def channel_middle():
            # ---- conv1 (st-pair outer) + c1T production ----
            with ExitStack() as c1ctx:
                wp = c1ctx.enter_context(tc.tile_pool(name="wp1c", bufs=4))
                bounce = c1ctx.enter_context(tc.tile_pool(name="bn1c",
                                                          bufs=3))
                tb4 = c1ctx.enter_context(tc.tile_pool(name="tb41c", bufs=2))
                wres4 = [load_wres(wp, w1_d.ap(), ob,
                                   (nc.sync, nc.scalar, nc.gpsimd,
                                    nc.sync)[ob])
                         for ob in range(NB)]
                load_xpad_bulk()
                for pair in range(NST // 2):
                    st0 = pair * 2
                    for ob in range(NB):
                        sb = conv1_pair(wres4[ob], ob, st0, bounce, b1_t)
                        c1t_out(sb, ob, st0, tb4)

            with ExitStack() as chctx:
                c1tp = chctx.enter_context(tc.tile_pool(name="c1tp", bufs=2))
                cattp = chctx.enter_context(tc.tile_pool(name="cattp",
                                                         bufs=NB))
                # G = c1 @ c1^T via transposed chunks
                pgt = [psL.tile([P, 1024], F32, tag="lg", name=f"pgt{i}")
                       for i in range(2)]
                pg = [pgt[cb // 2][:, (cb % 2) * 512:(cb % 2 + 1) * 512]
                      for cb in range(NB)]
                for j2 in range(NCH // 2):
                    c1t = c1tp.tile([P, 1024], F32R, tag="c1t", name="c1tin")
                    nc.sync.dma_start(
                        c1t[:].rearrange("p (j n) -> p j n", n=512),
                        c1t_d.ap()[j2 * 2:j2 * 2 + 2].rearrange(
                            "j p n -> p j n"))
                    for jj in range(2):
                        j = j2 * 2 + jj
                        ch = c1t[:, jj * 512:(jj + 1) * 512]
                        for cb in range(NB):
                            nc.tensor.matmul(pg[cb],
                                             ch[:, cb * P:(cb + 1) * P],
                                             ch[:], start=(j == 0),
                                             stop=(j == NCH - 1))
                catt = []
                for cb in range(NB):
                    negmax = statp.tile([P, 1], F32, tag="st", name="negmax")
                    nc.vector.reduce_max(negmax[:], pg[cb], axis=AX.X,
                                         negate=True)
                    ct = cattp.tile([P, 512], F32R, tag="ct",
                                    name=f"catt{cb}")
                    rowsum = statp.tile([P, 1], F32, tag="st", name="rowsum")
                    nc.scalar.activation(ct[:], pg[cb], AF.Exp,
                                         bias=negmax[:], accum_out=rowsum[:])
                    recip = statp.tile([P, 1], F32, tag="st", name="recip")
                    nc.vector.reciprocal(recip[:], rowsum[:])
                    # fold beta in: catt = beta * softmax(G)
                    nc.vector.tensor_mul(recip[:], recip[:], beta_t[:])
                    nc.scalar.activation(ct[:], ct[:], AF.Identity,
                                         scale=recip[:])
                    catt.append(ct)
                for st in range(NST):
                    c1s = c1tp.tile([P, NB, 512], F32R, tag="c4", name="c1s")
                    nc.sync.dma_start(
                        c1s[:],
                        s1_d.ap()[:, :, st * 512:(st + 1) * 512].rearrange(
                            "b p n -> p b n"))
                    for kb in range(NB):
                        pc = psA.tile([P, 512], F32, tag="mm", name="pc")
                        for cb in range(NB):
                            nc.tensor.matmul(
                                pc[:], catt[cb][:, kb * P:(kb + 1) * P],
                                c1s[:, cb], start=(cb == 0),
                                stop=(cb == NB - 1))
                        nc.vector.tensor_add(
                            _pad_view(xpad[kb][:], st), pc[:], c1s[:, kb])
            conv2()

        def conv2():
            # st-outer so it can chase the middle's residual writes
            with ExitStack() as c2ctx:
                wp = c2ctx.enter_context(tc.tile_pool(name="wp2", bufs=4))
                bounce2 = c2ctx.enter_context(tc.tile_pool(name="bn2",
                                                           bufs=2))
                wres4 = [load_wres(wp, w2_d.ap(), ob,
                                   (nc.sync, nc.scalar, nc.gpsimd,
                                    nc.sync)[ob])
                         for ob in range(NB)]
                for pair in range(NST // 2):
                    st0 = pair * 2
                    for ob in range(NB):
                        wres = wres4[ob]
                        ps = psL.tile([P, 1024], F32, tag="lg",
                                      name="c2p")
                        for tci in range(36):
                            cb, tap = tci // 9, tci % 9
                            dy, dx = tap // 3, tap % 3
                            for sl in range(2):
                                nc.tensor.matmul(
                                    ps[:, sl * 512:(sl + 1) * 512],
                                    wres[:, tci * P:(tci + 1) * P],
                                    _pad_view(xpad[cb][:], st0 + sl, dy, dx),
                                    start=(tci == 0), stop=(tci == 35))
                        sb = bounce2.tile([P, 1024], F32, tag="bn",
                                          name=f"ob{ob}")
                        nc.scalar.activation(sb[:], ps[:],
                                             AF.Relu, bias=b2_t[ob][:])
                        nc.gpsimd.dma_start(
                            out_d[ob, :, st0 * 512:(st0 + 2) * 512], sb[:])

        for _rep in range(reps):
            load_xpad()
            if branch == "spatial":
                spatial_middle()
            elif branch == "channel":
                channel_middle()
            else:
                pid = nc.partition_id()
                with tc.If(pid < 4) as cmp:
                    spatial_middle()
                with cmp.Else():
                    channel_middle()

        gctx.close()

    nc.compile()
    return nc


def _fold_conv(w, g, b, m, v):
    scale = g / np.sqrt(v + EPS)
    wf = (w * scale[:, None, None, None]).astype(np.float32)
    bf = (b - m * scale).astype(np.float32)
    # [O, CI, 3, 3] -> [ob, (cb tap), ci, o]
    wt = wf.transpose(2, 3, 1, 0).reshape(9, NB, P, NB, P).transpose(
        3, 1, 0, 2, 4).reshape(NB, 36, P, P)
    return np.ascontiguousarray(wt), bf.reshape(NB, P, 1)


def _pad_x(x):
    # x: [C, H, W] -> [NB, P, PAD]
    xp = np.zeros((NB, P, PR, PW), np.float32)
    xp[:, :, 1:65, 1:65] = x.reshape(NB, P, H, W)
    return xp.reshape(NB, P, PAD)


def prep_inputs(inputs):
    """Build the 8 per-core input maps from the full problem inputs."""
    x = np.asarray(inputs["x"], np.float32)
    alpha = float(np.asarray(inputs["alpha"]).reshape(-1)[0])
    beta = float(np.asarray(inputs["beta"]).reshape(-1)[0])

    w1s, b1s = _fold_conv(np.asarray(inputs["sa_w1"]), inputs["sa_g1"],
                          inputs["sa_b1"], inputs["sa_m1"], inputs["sa_v1"])
    w2s, b2s = _fold_conv(np.asarray(inputs["sa_w2"]), inputs["sa_g2"],
                          inputs["sa_b2"], inputs["sa_m2"], inputs["sa_v2"])
    w1c, b1c = _fold_conv(np.asarray(inputs["ca_w1"]), inputs["ca_g1"],
                          inputs["ca_b1"], inputs["ca_m1"], inputs["ca_v1"])
    w2c, b2c = _fold_conv(np.asarray(inputs["ca_w2"]), inputs["ca_g2"],
                          inputs["ca_b2"], inputs["ca_m2"], inputs["ca_v2"])

    qw = np.ascontiguousarray(np.asarray(inputs["q_w"], np.float32).T.reshape(
        NB, P, CI))
    kw = np.ascontiguousarray(np.asarray(inputs["k_w"], np.float32).T.reshape(
        NB, P, CI))
    vw = np.ascontiguousarray(
        (alpha * np.asarray(inputs["v_w"], np.float32)).T.reshape(NB, P, 512))
    qb = np.asarray(inputs["q_b"], np.float32).reshape(CI, 1)
    kb = np.asarray(inputs["k_b"], np.float32).reshape(CI, 1)
    vba = (alpha * np.asarray(inputs["v_b"], np.float32)).reshape(NB, P, 1)
    betat = np.full((P, 1), beta, np.float32)
    identr = np.eye(P, dtype=np.float32)
    import ml_dtypes
    identb = np.eye(P, dtype=ml_dtypes.bfloat16)

    zeros_qw = np.zeros_like(qw)
    zeros_vw = np.zeros_like(vw)
    zeros_b = np.zeros_like(qb)
    zeros_vba = np.zeros_like(vba)

    import ml_dtypes as _md
    w2sb = w2s.astype(_md.bfloat16)
    w2cb = w2c.astype(_md.bfloat16)
    maps = []
    for core in range(8):
        b = core % 4
        xp = _pad_x(x[b])
        if core < 4:
            m = dict(xpad=xp, w1=w1s, b1=b1s, w2=w2s, w2b=w2sb, b2=b2s,
                     qw=qw, kw=kw, vw=vw, qb=qb, kb=kb, vba=vba, betat=betat,
                     identr=identr, identb=identb)
        else:
            m = dict(xpad=xp, w1=w1c, b1=b1c, w2=w2c, w2b=w2cb, b2=b2c,
                     qw=zeros_qw, kw=zeros_qw, vw=zeros_vw, qb=zeros_b,
                     kb=zeros_b, vba=zeros_vba, betat=betat,
                     identr=identr, identb=identb)
        maps.append(m)
    return maps


def kernel(**inputs):
    if "nc" not in _CACHE:
        _CACHE["nc"] = build()
    nc = _CACHE["nc"]
    maps = prep_inputs(inputs)
    res = run_bass_kernel_spmd(nc, maps, core_ids=list(range(8)))
    out = np.zeros((B, C, H, W), np.float32)
    for b in range(B):
        sa = res.results[b]["out"].reshape(C, H, W)
        ca = res.results[b + 4]["out"].reshape(C, H, W)
        out[b] = sa + ca
    return out



# revision 72
# speedup vs baseline: 1.0047x; 1.0047x over previous
"""DualAttention2d Trainium2 kernel.

Sharding: 8 cores = 4 samples x {spatial-attention branch, channel-attention
branch}. Core c < 4 computes the spatial branch of sample c; core c >= 4
computes the channel branch of sample c-4. Host sums the two branch outputs.

Single SPMD program; branch divergence via tc.If(partition_id < 4).

Layout notes:
- Feature maps on-chip as [4 blocks][128 chan, S] with S = 64*64 = 4096.
- Conv inputs live in a zero-padded [128, 66*66] buffer (1-px halo); a 3x3
  conv is 9 shifted matmuls accumulated in PSUM over 4 channel blocks, two
  512-wide s-tiles per [128,1024] PSUM pair-tile, conv weights SBUF-resident
  for the whole conv (loaded once, split across DMA queues).
- BN is folded into conv weights/bias on the host; alpha into the v-proj.
- f32r matmuls run at full PE rate (free dim >= 512). The o = v @ att^T
  matmul runs in fp8e4 DoubleRow (256-deep k pairs, 2x PE rate): vT is
  produced fp8 directly by the conv1-fused v projection and kept SBUF-
  resident; attention probabilities (bf16, unnormalized) are transposed by
  PE in 4x[128,128] batches and evicted to the fp8 attT by DVE/Act.
  End-to-end fp8 error ~1.7e-3 (tolerance 2e-2).
- Softmax: logits pairs staged PSUM->SBUF f32 (copies 1 DVE / 3 Act),
  one fused negated row-max over the staged [128,4096] logits on DVE,
  single exp per 128-query block on Act with accumulated row-sum. Probabilities stay UNNORMALIZED; 1/rowsum is folded
  into the o eviction via a DRAM-roundtrip transpose+partition-broadcast
  of the 4 recip columns, applied as a [128,512] columnwise multiply, with
  the v-bias and residual fused in one scalar_tensor_tensor.
- Software pipelining: PE transposes of block b-1 are emitted between the
  logits matmuls of block b, and the o-matmuls of group g-1 are emitted
  under the early softmax blocks of group g, so PE never waits on the
  softmax chain. Startup: only conv pair 0's input rows load up front; the
  bulk of xpad queues behind the conv weight loads.
- HW quirks honored: GpSimd/Pool cannot touch PSUM (evictions on DVE/Act
  only); f32r cannot mix with bf16 matmul operands; DMA-issue SEQ slices
  span the transfer (big DMAs split across SP/Pool queues).
- TimelineSim: spatial ~0.95 ms, channel ~0.69 ms per core (baseline
  1.38/0.72); measured HW rel err 1.67e-3.
"""

import numpy as np

import concourse.bacc as bacc
import concourse.mybir as mybir
import concourse.tile as tile
from concourse.bass_utils import run_bass_kernel_spmd

B, C, H, W = 4, 512, 64, 64
S = H * W            # 4096
CI = 64              # q/k channels
P = 128
NB = C // P          # 4 channel blocks
PW = 66              # padded row width
PR = 66              # padded rows (1 zero row top/bottom)
PAD = PW * PR        # 4356
NST = S // 512       # 8 s-tiles of 512
NCH = S // P         # 32 s-chunks of 128
EPS = 1e-5

F32 = mybir.dt.float32
F32R = mybir.dt.float32r
BF16 = mybir.dt.bfloat16
F8 = mybir.dt.float8e4
DR = mybir.MatmulPerfMode.DoubleRow
AF = mybir.ActivationFunctionType
AX = mybir.AxisListType

_CACHE = {}


def _pad_view(xpad_ap, st, dy=1, dx=1):
    """View of padded buffer [128, PAD] covering s-tile `st` (8 image rows x 64
    cols) shifted by tap (dy, dx) in {0,1,2}^2. dy=dx=1 is the centered view."""
    v = xpad_ap.rearrange("p (r w) -> p r w", w=PW)
    r0 = st * 8 + dy
    return v[:, r0:r0 + 8, dx:dx + 64]


def build(branch=None, reps=1):
    """branch=None: SPMD program with If/Else on partition id.
    branch="spatial"/"channel": single-branch program (analysis/timing).
    reps>1 repeats the whole computation (for HW timing: per-rep exec =
    (wall[R] - wall[1]) / (R - 1), subtracting dispatch overhead)."""
    nc = bacc.Bacc("TRN2", target_bir_lowering=False, debug=False,
                   num_devices=8)

    # ---- I/O ----
    x_d = nc.dram_tensor("xpad", [NB, P, PAD], F32R, kind="ExternalInput")
    # conv weights pre-arranged host-side: [ob, tap, cb, ci, o]
    w1_d = nc.dram_tensor("w1", [NB, 36, P, P], F32R, kind="ExternalInput")
    b1_d = nc.dram_tensor("b1", [NB, P, 1], F32, kind="ExternalInput")
    w2_d = nc.dram_tensor("w2", [NB, 36, P, P], F32R, kind="ExternalInput")
    b2_d = nc.dram_tensor("b2", [NB, P, 1], F32, kind="ExternalInput")
    qw_d = nc.dram_tensor("qw", [NB, P, CI], F32R, kind="ExternalInput")
    kw_d = nc.dram_tensor("kw", [NB, P, CI], F32R, kind="ExternalInput")
    vw_d = nc.dram_tensor("vw", [NB, P, 512], F32R, kind="ExternalInput")
    qb_d = nc.dram_tensor("qb", [CI, 1], F32, kind="ExternalInput")
    kb_d = nc.dram_tensor("kb", [CI, 1], F32, kind="ExternalInput")
    vba_d = nc.dram_tensor("vba", [NB, P, 1], F32, kind="ExternalInput")
    beta_d = nc.dram_tensor("betat", [P, 1], F32, kind="ExternalInput")
    idr_d = nc.dram_tensor("identr", [P, P], F32R, kind="ExternalInput")
    idb_d = nc.dram_tensor("identb", [P, P], BF16, kind="ExternalInput")
    out_d = nc.dram_tensor("out", [NB, P, S], F32, kind="ExternalOutput")

    # ---- internal DRAM scratch ----
    s1_d = nc.dram_tensor("s1f", [NB, P, S], F32R, kind="Internal")
    c1t_d = nc.dram_tensor("c1t", [NCH, P, 512], F32R, kind="Internal")
    q_d = nc.dram_tensor("qs", [CI, S], F32R, kind="Internal")
    r_d = nc.dram_tensor("rcp", [2, 4, P], F32R, kind="Internal")

    with tile.TileContext(nc) as tc:
        from contextlib import ExitStack

        # ---- global pools (whole kernel) ----
        gctx = ExitStack()
        psA = gctx.enter_context(tc.tile_pool(name="psA", bufs=2,
                                              space="PSUM"))
        psL = gctx.enter_context(tc.tile_pool(name="psL", bufs=2,
                                              space="PSUM"))
        xpadp = gctx.enter_context(tc.tile_pool(name="xpadp", bufs=NB))
        consts = gctx.enter_context(tc.tile_pool(name="consts", bufs=1))
        b512 = gctx.enter_context(tc.tile_pool(name="b512", bufs=2))
        statp = gctx.enter_context(tc.tile_pool(name="statp", bufs=12))

        # ---- constants ----
        ident_r = consts.tile([P, P], F32R, name="ident_r")
        nc.sync.dma_start(ident_r[:], idr_d.ap())
        ident_b = consts.tile([P, P], BF16, name="ident_b")
        nc.sync.dma_start(ident_b[:], idb_d.ap())
        b1_t = [consts.tile([P, 1], F32, name=f"b1{i}") for i in range(NB)]
        b2_t = [consts.tile([P, 1], F32, name=f"b2{i}") for i in range(NB)]
        vba_t = [consts.tile([P, 1], F32, name=f"vba{i}") for i in range(NB)]
        qb_t = consts.tile([CI, 1], F32, name="qbt")
        kb_t = consts.tile([CI, 1], F32, name="kbt")
        beta_t = consts.tile([P, 1], F32, name="betat_sb")
        for i in range(NB):
            nc.sync.dma_start(b1_t[i][:], b1_d[i])
            nc.sync.dma_start(b2_t[i][:], b2_d[i])
            nc.sync.dma_start(vba_t[i][:], vba_d[i])
        nc.sync.dma_start(qb_t[:], qb_d.ap())
        nc.sync.dma_start(kb_t[:], kb_d.ap())
        nc.sync.dma_start(beta_t[:], beta_d.ap())

        # ---- padded input (loaded per rep) ----
        xpad = []

        def load_xpad():
            # only the rows conv pair 0 needs; bulk deferred behind the
            # conv weight loads (load_xpad_bulk) so PE starts ~10us in
            xpad.clear()
            xpad.extend(xpadp.tile([P, PAD], F32R, tag="xp", name=f"xpad{i}")
                        for i in range(NB))
            cut0 = 18 * PW
            for i in range(NB):
                (nc.sync, nc.gpsimd)[i % 2].dma_start(
                    xpad[i][:, :cut0], x_d[i, :, :cut0])

        def load_xpad_bulk():
            cut0 = 18 * PW
            cut1 = 34 * PW
            cut2 = 50 * PW
            for i in range(NB):
                nc.sync.dma_start(xpad[i][:, cut0:cut1], x_d[i, :, cut0:cut1])
                nc.gpsimd.dma_start(xpad[i][:, cut1:cut2],
                                    x_d[i, :, cut1:cut2])
                nc.sync.dma_start(xpad[i][:, cut2:], x_d[i, :, cut2:])

        def load_wres(wpool, w_dram, ob, eng=None):
            """The 36 [128,128] stationaries of one conv output block."""
            wres = wpool.tile([P, 36 * P], F32R, tag="wres", name="wres")
            (eng or nc.sync).dma_start(
                wres[:].rearrange("p (k o) -> p k o", o=P),
                w_dram[ob].rearrange("k p o -> p k o"))
            return wres

        def conv1_pair(wres, ob, st0, bounce, b1ref):
            """One conv over s-tiles (st0, st0+1) for output block ob; returns
            the evicted [128,1024] relu tile; also writes s1_d."""
            ps = psL.tile([P, 1024], F32, tag="lg", name="c1p")
            for tci in range(36):
                cb, tap = tci // 9, tci % 9
                dy, dx = tap // 3, tap % 3
                for sl in range(2):
                    nc.tensor.matmul(
                        ps[:, sl * 512:(sl + 1) * 512],
                        wres[:, tci * P:(tci + 1) * P],
                        _pad_view(xpad[cb][:], st0 + sl, dy, dx),
                        start=(tci == 0), stop=(tci == 35))
            sb = bounce.tile([P, 1024], F32R, tag="bn", name=f"sb{ob}")
            nc.scalar.activation(sb[:], ps[:], AF.Relu, bias=b1ref[ob][:])
            nc.gpsimd.dma_start(
                s1_d[ob, :, st0 * 512:(st0 + 2) * 512], sb[:])
            return sb

        def c1t_out(sb, ob, st0, tb4):
            """Transpose the pair-tile into c1t_d chunks (8 chunks)."""
            for sl in range(2):
                tb = tb4.tile([P, 512], F32R, tag="t4", name="tb")
                pt = psA.tile([P, 512], F32R, tag="mm", name="ptc")
                for j in range(4):
                    nc.tensor.transpose(
                        pt[:, j * P:(j + 1) * P],
                        sb[:, sl * 512 + j * P:sl * 512 + (j + 1) * P],
                        ident_r[:])
                nc.scalar.activation(tb[:], pt[:], AF.Identity)
                st = st0 + sl
                nc.gpsimd.dma_start(
                    c1t_d.ap()[st * 4:st * 4 + 4, :, ob * P:(ob + 1) * P]
                    .rearrange("j p c -> p j c"),
                    tb[:].rearrange("p (j c) -> p j c", c=P))

        def spatial_middle():
            # long-lived attention inputs: k global (f32r) + vT global (fp8)
            resctx = ExitStack()
            kqp = resctx.enter_context(tc.tile_pool(name="kqp", bufs=1))
            kg = kqp.tile([CI, S], F32R, tag="kg", name="kg")
            vt_sb = kqp.tile([P, NCH, 512], F8, tag="vt", name="vt_sb")

            # ---- conv1 fused with q/k/vT production, st-pair outer ----
            with ExitStack() as c1ctx:
                wp = c1ctx.enter_context(tc.tile_pool(name="wp1", bufs=4))
                bounce = c1ctx.enter_context(tc.tile_pool(name="bn1", bufs=4))
                qkvp = c1ctx.enter_context(tc.tile_pool(name="qkvp", bufs=1))
                qw_t = [qkvp.tile([P, CI], F32R, tag=f"qw{i}", name=f"qw{i}")
                        for i in range(NB)]
                kw_t = [qkvp.tile([P, CI], F32R, tag=f"kw{i}", name=f"kw{i}")
                        for i in range(NB)]
                vw_t = [qkvp.tile([P, 512], F32R, tag=f"vw{i}",
                                  name=f"vw{i}") for i in range(NB)]
                for i in range(NB):
                    nc.gpsimd.dma_start(qw_t[i][:], qw_d[i])
                    nc.gpsimd.dma_start(kw_t[i][:], kw_d[i])
                    nc.gpsimd.dma_start(vw_t[i][:], vw_d[i])
                wres4 = [load_wres(wp, w1_d.ap(), ob,
                                   (nc.sync, nc.gpsimd)[ob % 2])
                         for ob in range(NB)]
                load_xpad_bulk()
                for pair in range(NST // 2):
                    st0 = pair * 2
                    sbs = []
                    for ob in range(NB):
                        sb = conv1_pair(wres4[ob], ob, st0, bounce, b1_t)
                        sbs.append(sb)
                    # q, k, vT for the two s-tiles of this pair
                    for sl in range(2):
                        st = st0 + sl
                        ssl = slice(sl * 512, (sl + 1) * 512)
                        pq = psA.tile([CI, 512], F32, tag="mm", name="pq")
                        pk = psA.tile([CI, 512], F32, tag="mm", name="pk")
                        for cb in range(NB):
                            nc.tensor.matmul(pq[:], qw_t[cb][:],
                                             sbs[cb][:, ssl],
                                             start=(cb == 0),
                                             stop=(cb == NB - 1))
                        for cb in range(NB):
                            nc.tensor.matmul(pk[:], kw_t[cb][:],
                                             sbs[cb][:, ssl],
                                             start=(cb == 0),
                                             stop=(cb == NB - 1))
                        qsb = b512.tile([CI, 512], F32R, tag="bn",
                                        name="qsb")
                        nc.scalar.activation(qsb[:], pq[:], AF.Identity,
                                             bias=qb_t[:])
                        nc.gpsimd.dma_start(
                            q_d.ap()[:, st * 512:(st + 1) * 512], qsb[:])
                        nc.scalar.activation(kg[:, st * 512:(st + 1) * 512],
                                             pk[:], AF.Identity, bias=kb_t[:])
                        for j in range(4):
                            pv = psA.tile([P, 512], F32, tag="mm", name="pv")
                            for cb in range(NB):
                                nc.tensor.matmul(
                                    pv[:],
                                    sbs[cb][:, sl * 512 + j * P:
                                            sl * 512 + (j + 1) * P],
                                    vw_t[cb][:], start=(cb == 0),
                                    stop=(cb == NB - 1))
                            nc.scalar.activation(
                                vt_sb[:, st * 4 + j, :], pv[:], AF.Identity)

            # ---- attention, one group of 512 query positions at a time ----
            with ExitStack() as attctx:
                qgp = attctx.enter_context(tc.tile_pool(name="qgp", bufs=1))

                logp = attctx.enter_context(tc.tile_pool(name="logp", bufs=2))
                probp = attctx.enter_context(tc.tile_pool(name="probp",
                                                          bufs=2))
                attTp = attctx.enter_context(tc.tile_pool(name="attTp",
                                                          bufs=2))
                rcbp = attctx.enter_context(tc.tile_pool(name="rcbp", bufs=2))
                s1rp = attctx.enter_context(tc.tile_pool(name="s1rp", bufs=2))
                def o_phase(attT, recipb, g):
                    # o = vT^T @ attT via fp8 DoubleRow (256-deep k pairs)
                    for half in range(2):
                        cbs = (2 * half, 2 * half + 1)
                        s1r = s1rp.tile([P, 1024], F32R, tag="s1r",
                                        name="s1r")
                        nc.sync.dma_start(
                            s1r[:].rearrange("p (b n) -> p b n", n=512),
                            s1_d.ap()[2 * half:2 * half + 2, :,
                                      g * 512:(g + 1) * 512].rearrange(
                                          "b p n -> p b n"))
                        pop = psL.tile([P, 1024], F32, tag="lg", name="po")
                        po = [pop[:, :512], pop[:, 512:]]
                        for jp in range(NCH // 2):
                            for i, cb in enumerate(cbs):
                                nc.tensor.matmul(
                                    po[i],
                                    vt_sb[:, 2 * jp:2 * jp + 2,
                                          cb * P:(cb + 1) * P],
                                    attT[:, 2 * jp:2 * jp + 2, :],
                                    start=(jp == 0), stop=(jp == NCH // 2 - 1),
                                    perf_mode=DR)
                        for i, cb in enumerate(cbs):
                            ob_sb = b512.tile([P, 512], F32, tag="bn",
                                              name="obsb")
                            nc.vector.tensor_mul(ob_sb[:], po[i],
                                                 recipb[:])
                            nc.vector.scalar_tensor_tensor(
                                _pad_view(xpad[cb][:], g), ob_sb[:],
                                vba_t[cb][:], s1r[:, i * 512:(i + 1) * 512],
                                op0=mybir.AluOpType.add,
                                op1=mybir.AluOpType.add)

                prev = None
                for g in range(NST):
                    qg = qgp.tile([CI, 512], F32R, tag="qg", name="qg")
                    nc.sync.dma_start(qg[:],
                                      q_d.ap()[:, g * 512:(g + 1) * 512])
                    attT = attTp.tile([P, NCH, 512], F8, tag="attT",
                                      name=f"attT{g}")
                    recip4 = statp.tile([P, 4], F32R, tag="rc", name="recip4")
                    rowsum4 = statp.tile([P, 4], F32, tag="rs",
                                         name="rowsum4")
                    probs4 = []

                    def softmax_blk(blk):
                        """logits matmuls + softmax for one 128-query block;
                        probs left UNNORMALIZED (recip folded into o-evict)."""
                        logits = logp.tile([P, S], F32, tag="lg",
                                           name="logits")
                        for sp in range(NST // 2):
                            pl = psL.tile([P, 1024], F32, tag="lg", name="pl")
                            for sl in range(2):
                                st = sp * 2 + sl
                                nc.tensor.matmul(
                                    pl[:, sl * 512:(sl + 1) * 512],
                                    qg[:, blk * P:(blk + 1) * P],
                                    kg[:, st * 512:(st + 1) * 512],
                                    start=True, stop=True)
                            if sp == 0:
                                nc.vector.tensor_copy(
                                    logits[:, sp * 1024:(sp + 1) * 1024],
                                    pl[:])
                            else:
                                nc.scalar.copy(
                                    logits[:, sp * 1024:(sp + 1) * 1024],
                                    pl[:])
                        negmax = statp.tile([P, 1], F32, tag="st",
                                            name="negmax")
                        nc.vector.reduce_max(negmax[:], logits[:], axis=AX.X,
                                             negate=True)
                        probs = probp.tile([P, S], BF16, tag="pb",
                                           name="probs")
                        nc.scalar.activation(probs[:], logits[:], AF.Exp,
                                             bias=negmax[:],
                                             accum_out=rowsum4[:,
                                                              blk:blk + 1])
                        return probs

                    def transpose_blk(blk):
                        probs = probs4[blk]
                        for j4 in range(NCH // 4):
                            pt = psA.tile([P, 512], BF16, tag="tb",
                                          name="pt")
                            for jj in range(4):
                                j = j4 * 4 + jj
                                nc.tensor.transpose(
                                    pt[:, jj * P:(jj + 1) * P],
                                    probs[:, j * P:(j + 1) * P],
                                    ident_b[:])
                            dst = attT[:, j4 * 4:j4 * 4 + 4,
                                       blk * P:(blk + 1) * P]
                            srcv = pt[:].rearrange("p (j q) -> p j q", q=P)
                            if j4 % 2 == 0:
                                nc.vector.tensor_copy(dst, srcv)
                            else:
                                nc.scalar.copy(dst, srcv)

                    # software pipeline: transposes of blk-1 overlap blk's
                    # softmax chain on Act/DVE; the o-matmuls of g-1 fill
                    # the PE idle under the early blocks' softmax chains
                    for blk in range(4):
                        probs4.append(softmax_blk(blk))
                        if blk >= 1:
                            transpose_blk(blk - 1)
                        if blk == 1 and prev is not None:
                            o_phase(*prev)
                    transpose_blk(3)

                    # recip row via DRAM roundtrip: [128q,4] -> [1,512]
                    # transposed view -> partition-broadcast load [128,512]
                    with nc.allow_low_precision(reason="f32r==f32 bits"):
                        nc.vector.reciprocal(recip4[:], rowsum4[:])
                    nc.sync.dma_start(
                        r_d[g % 2].rearrange("b q -> q b"), recip4[:])
                    recipb = rcbp.tile([P, 512], F32R, tag="rb",
                                       name="recipb")
                    nc.sync.dma_start(
                        recipb[:],
                        r_d.ap()[g % 2:g % 2 + 1].rearrange(
                            "o b q -> o (b q)").to_broadcast((P, 512)))

                    prev = (attT, recipb, g)
                o_phase(*prev)
            resctx.close()
            conv2()

        def channel_middle():
            # ---- conv1 (st-pair outer) + c1T production ----
            with ExitStack() as c1ctx:
                wp = c1ctx.enter_context(tc.tile_pool(name="wp1c", bufs=4))
                bounce = c1ctx.enter_context(tc.tile_pool(name="bn1c",
                                                          bufs=3))
                tb4 = c1ctx.enter_context(tc.tile_pool(name="tb41c", bufs=2))
                wres4 = [load_wres(wp, w1_d.ap(), ob,
                                   (nc.sync, nc.scalar, nc.gpsimd,
                                    nc.sync)[ob])
                         for ob in range(NB)]
                load_xpad_bulk()
                for pair in range(NST // 2):
                    st0 = pair * 2
                    for ob in range(NB):
                        sb = conv1_pair(wres4[ob], ob, st0, bounce, b1_t)
                        c1t_out(sb, ob, st0, tb4)

            with ExitStack() as chctx:
                c1tp = chctx.enter_context(tc.tile_pool(name="c1tp", bufs=2))
                cattp = chctx.enter_context(tc.tile_pool(name="cattp",
                                                         bufs=NB))
                # G = c1 @ c1^T via transposed chunks
                pgt = [psL.tile([P, 1024], F32, tag="lg", name=f"pgt{i}")
                       for i in range(2)]
                pg = [pgt[cb // 2][:, (cb % 2) * 512:(cb % 2 + 1) * 512]
                      for cb in range(NB)]
                for j2 in range(NCH // 2):
                    c1t = c1tp.tile([P, 1024], F32R, tag="c1t", name="c1tin")
                    nc.sync.dma_start(
                        c1t[:].rearrange("p (j n) -> p j n", n=512),
                        c1t_d.ap()[j2 * 2:j2 * 2 + 2].rearrange(
                            "j p n -> p j n"))
                    for jj in range(2):
                        j = j2 * 2 + jj
                        ch = c1t[:, jj * 512:(jj + 1) * 512]
                        for cb in range(NB):
                            nc.tensor.matmul(pg[cb],
                                             ch[:, cb * P:(cb + 1) * P],
                                             ch[:], start=(j == 0),
                                             stop=(j == NCH - 1))
                catt = []
                for cb in range(NB):
                    negmax = statp.tile([P, 1], F32, tag="st", name="negmax")
                    nc.vector.reduce_max(negmax[:], pg[cb], axis=AX.X,
                                         negate=True)
                    ct = cattp.tile([P, 512], F32R, tag="ct",
                                    name=f"catt{cb}")
                    rowsum = statp.tile([P, 1], F32, tag="st", name="rowsum")
                    nc.scalar.activation(ct[:], pg[cb], AF.Exp,
                                         bias=negmax[:], accum_out=rowsum[:])
                    recip = statp.tile([P, 1], F32, tag="st", name="recip")
                    nc.vector.reciprocal(recip[:], rowsum[:])
                    # fold beta in: catt = beta * softmax(G)
                    nc.vector.tensor_mul(recip[:], recip[:], beta_t[:])
                    nc.scalar.activation(ct[:], ct[:], AF.Identity,
                                         scale=recip[:])
                    catt.append(ct)
                for st in range(NST):
                    c1s = c1tp.tile([P, NB, 512], F32R, tag="c4", name="c1s")
                    nc.sync.dma_start(
                        c1s[:],
                        s1_d.ap()[:, :, st * 512:(st + 1) * 512].rearrange(
                            "b p n -> p b n"))
                    for kb in range(NB):
                        pc = psA.tile([P, 512], F32, tag="mm", name="pc")
                        for cb in range(NB):
                            nc.tensor.matmul(
                                pc[:], catt[cb][:, kb * P:(kb + 1) * P],
                                c1s[:, cb], start=(cb == 0),
                                stop=(cb == NB - 1))
                        nc.vector.tensor_add(
                            _pad_view(xpad[kb][:], st), pc[:], c1s[:, kb])
            conv2()

        def conv2():
            # st-outer so it can chase the middle's residual writes
            with ExitStack() as c2ctx:
                wp = c2ctx.enter_context(tc.tile_pool(name="wp2", bufs=4))
                bounce2 = c2ctx.enter_context(tc.tile_pool(name="bn2",
                                                           bufs=2))
                wres4 = [load_wres(wp, w2_d.ap(), ob,
                                   (nc.sync, nc.scalar, nc.gpsimd,
                                    nc.sync)[ob])
                         for ob in range(NB)]
                for pair in range(NST // 2):
                    st0 = pair * 2
                    for ob in range(NB):
                        wres = wres4[ob]
                        ps = psL.tile([P, 1024], F32, tag="lg",
                                      name="c2p")
                        for tci in range(36):
                            cb, tap = tci // 9, tci % 9
                            dy, dx = tap // 3, tap % 3
                            for sl in range(2):
                                nc.tensor.matmul(
                                    ps[:, sl * 512:(sl + 1) * 512],
                                    wres[:, tci * P:(tci + 1) * P],
                                    _pad_view(xpad[cb][:], st0 + sl, dy, dx),
                                    start=(tci == 0), stop=(tci == 35))
                        sb = bounce2.tile([P, 1024], F32, tag="bn",
                                          name=f"ob{ob}")
                        nc.scalar.activation(sb[:], ps[:],
                                             AF.Relu, bias=b2_t[ob][:])
                        nc.gpsimd.dma_start(
                            out_d[ob, :, st0 * 512:(st0 + 2) * 512], sb[:])

        for _rep in range(reps):
            load_xpad()
            if branch == "spatial":
                spatial_middle()
            elif branch == "channel":
                channel_middle()
            else:
                pid = nc.partition_id()
                with tc.If(pid < 4) as cmp:
                    spatial_middle()
                with cmp.Else():
                    channel_middle()

        gctx.close()

    nc.compile()
    return nc


def _fold_conv(w, g, b, m, v):
    scale = g / np.sqrt(v + EPS)
    wf = (w * scale[:, None, None, None]).astype(np.float32)
    bf = (b - m * scale).astype(np.float32)
    # [O, CI, 3, 3] -> [ob, (cb tap), ci, o]
    wt = wf.transpose(2, 3, 1, 0).reshape(9, NB, P, NB, P).transpose(
        3, 1, 0, 2, 4).reshape(NB, 36, P, P)
    return np.ascontiguousarray(wt), bf.reshape(NB, P, 1)


def _pad_x(x):
    # x: [C, H, W] -> [NB, P, PAD]
    xp = np.zeros((NB, P, PR, PW), np.float32)
    xp[:, :, 1:65, 1:65] = x.reshape(NB, P, H, W)
    return xp.reshape(NB, P, PAD)


def prep_inputs(inputs):
    """Build the 8 per-core input maps from the full problem inputs."""
    x = np.asarray(inputs["x"], np.float32)
    alpha = float(np.asarray(inputs["alpha"]).reshape(-1)[0])
    beta = float(np.asarray(inputs["beta"]).reshape(-1)[0])

    w1s, b1s = _fold_conv(np.asarray(inputs["sa_w1"]), inputs["sa_g1"],
                          inputs["sa_b1"], inputs["sa_m1"], inputs["sa_v1"])
    w2s, b2s = _fold_conv(np.asarray(inputs["sa_w2"]), inputs["sa_g2"],
                          inputs["sa_b2"], inputs["sa_m2"], inputs["sa_v2"])
    w1c, b1c = _fold_conv(np.asarray(inputs["ca_w1"]), inputs["ca_g1"],
                          inputs["ca_b1"], inputs["ca_m1"], inputs["ca_v1"])
    w2c, b2c = _fold_conv(np.asarray(inputs["ca_w2"]), inputs["ca_g2"],
                          inputs["ca_b2"], inputs["ca_m2"], inputs["ca_v2"])

    qw = np.ascontiguousarray(np.asarray(inputs["q_w"], np.float32).T.reshape(
        NB, P, CI))
    kw = np.ascontiguousarray(np.asarray(inputs["k_w"], np.float32).T.reshape(
        NB, P, CI))
    vw = np.ascontiguousarray(
        (alpha * np.asarray(inputs["v_w"], np.float32)).T.reshape(NB, P, 512))
    qb = np.asarray(inputs["q_b"], np.float32).reshape(CI, 1)
    kb = np.asarray(inputs["k_b"], np.float32).reshape(CI, 1)
    vba = (alpha * np.asarray(inputs["v_b"], np.float32)).reshape(NB, P, 1)
    betat = np.full((P, 1), beta, np.float32)
    identr = np.eye(P, dtype=np.float32)
    import ml_dtypes
    identb = np.eye(P, dtype=ml_dtypes.bfloat16)

    zeros_qw = np.zeros_like(qw)
    zeros_vw = np.zeros_like(vw)
    zeros_b = np.zeros_like(qb)
    zeros_vba = np.zeros_like(vba)

    import ml_dtypes as _md
    w2sb = w2s.astype(_md.bfloat16)
    w2cb = w2c.astype(_md.bfloat16)
    maps = []
    for core in range(8):
        b = core % 4
        xp = _pad_x(x[b])
        if core < 4:
            m = dict(xpad=xp, w1=w1s, b1=b1s, w2=w2s, w2b=w2sb, b2=b2s,
                     qw=qw, kw=kw, vw=vw, qb=qb, kb=kb, vba=vba, betat=betat,
                     identr=identr, identb=identb)
        else:
            m = dict(xpad=xp, w1=w1c, b1=b1c, w2=w2c, w2b=w2cb, b2=b2c,
                     qw=zeros_qw, kw=zeros_qw, vw=zeros_vw, qb=zeros_b,
                     kb=zeros_b, vba=zeros_vba, betat=betat,
                     identr=identr, identb=identb)
        maps.append(m)
    return maps


def kernel(**inputs):
    if "nc" not in _CACHE:
        _CACHE["nc"] = build()
    nc = _CACHE["nc"]
    maps = prep_inputs(inputs)
    res = run_bass_kernel_spmd(nc, maps, core_ids=list(range(8)))
    out = np.zeros((B, C, H, W), np.float32)
    for b in range(B):
        sa = res.results[b]["out"].reshape(C, H, W)
        ca = res.results[b + 4]["out"].reshape(C, H, W)
        out[b] = sa + ca
    return out

